# revision 40
# baseline (speedup 1.0000x reference)
"""Trainium2 Bass kernel for DenseEquivariantShiftModule (v2: pipelined pairs).

shift[b,i,c] = ( sum_k pb[b,i,k,c]*ps[b,i,k]
               + (1/A_b) sum_k sum_j u[b,j]*rb[b,i,j,k,c]*rs[b,i,j,k] ) / A_b
where ps = MLP_pw(pointwise_features), rs = MLP_rel(relative_features),
u = ~masked, A_b = sum_j u[b,j].

Sharding: B*N = 1024 "i" rows split across 8 cores (128 rows each, each
core within one batch element).

v2 schedule (per PAIR of i-rows, software-pipelined across stages):
  PE   : L1(p) [2 matmuls, 448 moving cols each, W1 stationary]
         L2(p-1) [8 chunk matmuls, H1 chunks stationary, W2 moving]
         MT(p-2) [8 matmuls, us-weighted rb stationary, H2 moving]
  ACT  : relu1(p) batched over the pair  (PSUM->SBUF, bias b1)
  DVE  : relu2(p-1) batched over the pair; mt*W3 multiply per 4-row group
  GPSIMD: reduce_sum over h per 4-row group -> rall[12, i]
  DMA  : xr trimmed to the 448 unmasked j columns (rb kills masked j).
Final: transpose rall, pointwise MLP, combine, out [128, 3] per core.
"""
import sys

sys.path.insert(0, "/opt/trn_rl_repo")

import ml_dtypes
import numpy as np

import concourse.bass as bass
import concourse.tile as tile
from concourse import masks, mybir

B, N, F, NB = 2, 512, 128, 4
NCORES = 8
IPC = B * N // NCORES  # i-rows per core
NCH = N // 128  # j-chunks per i-row
f32 = mybir.dt.float32
f32r = mybir.dt.float32r
bf16 = mybir.dt.bfloat16
fp8 = mybir.dt.float8e4


def _install_tile_patch():
    """walrus in this container accepts only 1 sem wait per CTRL
    instruction; TileContext's tail drain carries one per touched
    processor. Split them across SP NOPs."""
    import re

    import bass_rust
    from concourse.vector_clock import ScopedClock

    def _patched(self, tick_clock, wait_clock):
        gc = tick_clock.global_clock
        vals = eval(re.match(r"VectorClock\((\[.*\])\)", repr(gc)).group(1))
        for i, v in enumerate(vals):
            if v <= 0:
                continue
            sub = [0] * len(vals)
            sub[i] = v
            nop = self.nc.sync.nop(nofuse=True, hint="drain_wait_split")
            wait_clock.add_sem_waits(
                nop.ins, ScopedClock({None: bass_rust.VectorClock(sub)})
            )
        self.nc.sync.drain()
        self.nc.all_engine_barrier()
        assert self.sems is not None
        popped = self.nc._tile_sem_poison_stack.pop()
        assert popped is self._sem_poison
        self.nc.clear_and_free_semaphores(list(self.sems.allocated().values()))
        self.nc.all_engine_barrier()

    tile.TileContext._drain_and_barrier = _patched


def _split_multi_waits(nc):
    """This walrus build accepts a single sem wait per instruction.
    Move extra waits onto same-engine NOPs inserted just before the
    owning instruction (engine streams execute in block order, so the
    NOP's wait blocks the engine exactly as the fused wait would)."""
    import bass_rust

    n = 0
    for f in nc.m.functions:
        for bb in f.blocks:
            insts = bb.instructions
            i = 0
            while i < len(insts):
                ins = insts[i]
                si = ins.sync_info
                if si is not None and si.on_wait and len(si.on_wait) > 1:
                    waits = list(si.on_wait)
                    updates = list(si.on_update) if si.on_update else []
                    for w in waits[:-1]:
                        nop = mybir.InstNoOp(
                            name=f"I-waitsplit-{n}", ins=[], outs=[]
                        )
                        n += 1
                        nop.engine = ins.engine
                        nop.sync_info = bass_rust.SyncInfo(
                            on_wait=[w], on_update=[]
                        )
                        insts.insert(i, nop)
                        i += 1
                    ins.sync_info = bass_rust.SyncInfo(
                        on_wait=[waits[-1]], on_update=updates
                    )
                i += 1
    return n


def build_program(ipc=IPC, jt=448, split_waits=True):
    _install_tile_patch()
    nc = bass.Bass()
    xr = nc.dram_tensor("xr", [F, ipc, jt], fp8, kind="ExternalInput")
    w1f = nc.dram_tensor("w1f", [F, 128], fp8, kind="ExternalInput")
    rbm = nc.dram_tensor("rbm", [128, ipc, NCH * 12], bf16, kind="ExternalInput")
    # all f32 constants in one blob (single DMA, fat packets):
    # w3q 0:128 | ident 128:256 | pw1 256:384 | pw2 384:512 | pw3 512:516 |
    # pb1 516 | pb2 517 | pb3 518:522 | pbp 522:534 | xp 534:662 |
    # btm 662:674 | b1 674
    cstf = nc.dram_tensor("cstf", [128, 675], f32, kind="ExternalInput")
    cstb = nc.dram_tensor("cstb", [128, 128], bf16, kind="ExternalInput")
    out = nc.dram_tensor("out", [ipc, 3], f32, kind="ExternalOutput")

    from contextlib import ExitStack

    with tile.TileContext(nc) as tc:
        with ExitStack() as ctx:
            _kernel_body(ctx, tc, ipc, jt, xr, w1f, rbm, cstf, cstb, out)
    if split_waits:
        _split_multi_waits(nc)
    return nc


def _kernel_body(ctx, tc, ipc, jt, xr, w1f, rbm, cstf, cstb, out):
    nc = tc.nc
    Relu = mybir.ActivationFunctionType.Relu
    Copy = mybir.ActivationFunctionType.Copy

    consts = ctx.enter_context(tc.tile_pool(name="consts", bufs=1))
    xtpool = ctx.enter_context(tc.tile_pool(name="x", bufs=3))
    h1pool = ctx.enter_context(tc.tile_pool(name="h1", bufs=3))
    h2pool = ctx.enter_context(tc.tile_pool(name="h2", bufs=3))
    smallpool = ctx.enter_context(tc.tile_pool(name="small", bufs=4))
    ps_xt = ctx.enter_context(tc.tile_pool(name="ps_xt", bufs=1, space="PSUM"))
    ps_h1 = ctx.enter_context(tc.tile_pool(name="ps_h1", bufs=2, space="PSUM"))
    ps_h2 = ctx.enter_context(tc.tile_pool(name="ps_h2", bufs=2, space="PSUM"))
    ps_mt = ctx.enter_context(tc.tile_pool(name="ps_mt", bufs=1, space="PSUM"))

    assert ipc % 4 == 0
    NP = ipc // 2  # pairs
    nquads = (ipc + 3) // 4
    pr = min(ipc, 128)

    # xt quads first on the sync queue: the first L1 depends on quad 0, so
    # nothing may queue ahead of it
    xtq = {}

    def issue_quad(q):
        t = xtpool.tile([128, 4, jt], fp8, tag="xts")
        nc.sync.dma_start(out=t[:], in_=xr[:, 4 * q : 4 * q + 4, :])
        xtq[q] = t

    issue_quad(0)
    w1s_t = consts.tile([128, 128], fp8)
    nc.sync.dma_start(out=w1s_t[:], in_=w1f[:])
    cstb_sb = consts.tile([128, 128], bf16)
    nc.sync.dma_start(out=cstb_sb[:], in_=cstb[:])
    if nquads > 1:
        issue_quad(1)
    # rb (host pre-transposed to partition-major, 12KB contiguous runs)
    rb_all = consts.tile([128, ipc, NCH * 12], bf16)
    half = ipc // 2
    nc.gpsimd.dma_start(out=rb_all[:, 0:half, :], in_=rbm[:, 0:half, :])
    nc.gpsimd.dma_start(out=rb_all[:, half:ipc, :], in_=rbm[:, half:ipc, :])
    cstf_sb = consts.tile([128, 675], f32)
    nc.sync.dma_start(out=cstf_sb[:], in_=cstf[:])

    w1s = w1s_t[:]
    w2s = cstb_sb[:, 0:128]
    w3qs = cstf_sb[:, 0:128]
    ident = cstf_sb[:, 128:256]
    pw1s = cstf_sb[:, 256:384]
    pw2s = cstf_sb[:, 384:512]
    pw3s = cstf_sb[:, 512:516]
    pb1s = cstf_sb[:, 516:517]
    pb2s = cstf_sb[:, 517:518]
    pb3s = cstf_sb[:, 518:522]
    pbps = cstf_sb[:, 522:534]
    xp_sb = cstf_sb[:, 534:662]
    btm_sb = cstf_sb[:, 662:674]
    b1s = cstf_sb[:, 674:675]

    # rall2[32*r + kc, g] = j-reduced, W3-contracted result for row 4g+r
    rall2 = consts.tile([128, ipc // 4], f32)

    # zero the unwritten j-tail of the rotating h1 SBUF buffers once so the
    # (finite-garbage) tail columns can never inject NaN downstream
    h1sb_boot = []
    for _ in range(3):
        t = h1pool.tile([128, 2, 512], bf16, tag="h1s")
        if jt < 512:
            nc.vector.memset(t[:, :, jt:512], 0.0)
        h1sb_boot.append(t)

    st_h1sb = {}  # pair -> h1 sbuf tile
    st_h2ps = {}
    st_h2sb = {}
    mtps = {}  # group -> psum tile

    for p in range(NP + 2):
        # ---- stage A: DMA prefetch + L1 + relu1 for pair p ----
        if p < NP:
            q = p // 2
            if p % 2 == 0 and q + 2 < nquads:
                issue_quad(q + 2)
            xt = xtq[q]
            if p < 3:
                h1_sb = h1sb_boot[p]
            else:
                h1_sb = h1pool.tile([128, 2, 512], bf16, tag="h1s")
            for r in range(2):
                a = 2 * (p % 2) + r
                h1_ps = ps_h1.tile([128, 512], f32, tag="h1")
                nc.tensor.matmul(h1_ps[:, 0:jt], w1s, xt[:, a, :])
                # relu1 per-row on ACT; W1 is uploaded as 8*W1 in fp8,
                # the 1/8 rides the activation's free scale
                nc.scalar.activation(
                    h1_sb[:, r, 0:jt], h1_ps[:, 0:jt], Relu, bias=b1s,
                    scale=0.125,
                )
            if p % 2 == 1 and (q - 1) in xtq:
                del xtq[q - 1]
            st_h1sb[p] = h1_sb

        # ---- stage B: L2 + relu2 for pair p-1 ----
        pb_ = p - 1
        if 0 <= pb_ < NP:
            h1_sb = st_h1sb[pb_]
            h2_ps = ps_h2.tile([128, 2, NCH, 128], f32, tag="h2")
            for r in range(2):
                for c in range(NCH):
                    nc.tensor.matmul(
                        h2_ps[:, r, c, :],
                        h1_sb[:, r, c * 128 : (c + 1) * 128],
                        w2s,
                    )
            h2_sb = h2pool.tile([128, 2, NCH, 128], bf16, tag="h2s")
            # relu2 pair-batched on DVE (rel_b2 == 0)
            nc.vector.tensor_scalar(
                h2_sb[:].rearrange("p a c h -> p (a c h)"),
                h2_ps[:].rearrange("p a c h -> p (a c h)"),
                scalar1=0.0,
                scalar2=None,
                op0=mybir.AluOpType.max,
            )
            st_h2ps[pb_] = h2_ps
            st_h2sb[pb_] = h2_sb
            del st_h1sb[pb_]

        # ---- stage C: MT + group close for pair p-2 ----
        # 4 consecutive rows land in one PSUM bank at partition offsets
        # 0/32/64/96 (tile_position column groups); one fused TTR then
        # multiplies by W3 and reduces over h for all 4 rows at once.
        pc_ = p - 2
        if 0 <= pc_ < NP:
            h2_sb = st_h2sb[pc_]
            for r in range(2):
                ii = 2 * pc_ + r
                i4 = ii % 4
                g = ii // 4
                if i4 == 0:
                    mt_ps_new = ps_mt.tile([128, 128], f32, tag="mt")
                    mtps[g] = mt_ps_new
                mt_ps = mtps[g]
                for c in range(NCH):
                    nc.tensor.matmul(
                        mt_ps[32 * i4 : 32 * i4 + 12, :],
                        rb_all[:, ii, c * 12 : (c + 1) * 12],
                        h2_sb[:, r, c, :],
                        start=(c == 0),
                        stop=(c == NCH - 1),
                        tile_position=(0, 32 * i4),
                    )
                if i4 == 3:
                    # fused (mt * w3) + h-sum for 4 packed rows in one op
                    tmp = smallpool.tile([128, 128], f32, tag="tmp")
                    nc.vector.scalar_tensor_tensor(
                        tmp[:],
                        mt_ps[:],
                        1.0,
                        w3qs,
                        op0=mybir.AluOpType.mult,
                        op1=mybir.AluOpType.mult,
                        accum_out=rall2[:, g : g + 1],
                    )
            del st_h2sb[pc_]
            del st_h2ps[pc_]

        # ---- one-shot pointwise MLP, slotted into loop slack right after
        # group 1 closed (its ps_mt buffer is free until group 2 opens) ----
        if p == 5:
            xtp_ps = ps_xt.tile([128, 128], f32, tag="xt")
            nc.tensor.transpose(xtp_ps[:], xp_sb, ident)
            xtp_sb = xtpool.tile([128, 128], f32, tag="xts")
            nc.scalar.activation(xtp_sb[:], xtp_ps[:], Copy)
            h1p_ps = ps_h1.tile([128, 128], f32, tag="h1")
            nc.tensor.matmul(h1p_ps[:], pw1s, xtp_sb[:])
            h1p_sb = h1pool.tile([128, 128], f32, tag="h1s")
            nc.scalar.activation(h1p_sb[:], h1p_ps[:], Relu, bias=pb1s)
            h2p_ps = ps_h2.tile([128, 128], f32, tag="h2")
            nc.tensor.matmul(h2p_ps[:], pw2s, h1p_sb[:])
            h2p_sb = h2pool.tile([128, 128], f32, tag="h2s")
            nc.scalar.activation(h2p_sb[:], h2p_ps[:], Relu, bias=pb2s)
            psc_ps = ps_mt.tile([128, NB], f32, tag="mt")
            nc.tensor.matmul(psc_ps[:], h2p_sb[:], pw3s)
            psc_sb = consts.tile([128, NB], f32)
            nc.vector.tensor_add(psc_sb[:], psc_ps[:], pb3s)

    # unpack rall2 [32r+kc, g] -> rall [kc, 4g+r]
    rall = consts.tile([12, 128], f32)
    for r in range(4):
        dst = bass.AP(
            tensor=rall[:].tensor,
            offset=rall[:].offset + r,
            ap=[rall[:].ap[0], [4, ipc // 4]],
        )
        nc.vector.tensor_copy(dst, rall2[32 * r : 32 * r + 12, :])

    # transpose R [12, i] -> [i, 12]
    rsq_ps = ps_xt.tile([128, 128], f32, tag="xt")
    nc.tensor.transpose(rsq_ps[0:pr, 0:12], rall[0:12, 0:pr],
                        cstf_sb[0:12, 128:140])
    rsq = consts.tile([128, 12], f32)
    if pr < 128:
        nc.vector.memset(rsq[:], 0.0)
    nc.scalar.activation(rsq[0:pr, :], rsq_ps[0:pr, 0:12], Copy)

    # tot[i, kc] = rsq + pbp*ps_bcast + btm ; out = sum_k tot
    prodp = consts.tile([128, 12], f32)
    pb_v = pbps.rearrange("p (k c) -> p k c", k=NB)
    ps_v = bass.AP(
        tensor=psc_sb[:].tensor,
        offset=psc_sb[:].offset,
        ap=[psc_sb[:].ap[0], [1, NB], [0, 3]],
    )
    prodp_v = prodp[:].rearrange("p (k c) -> p k c", k=NB)
    nc.vector.tensor_mul(prodp_v, pb_v, ps_v)
    tot = consts.tile([128, 12], f32)
    nc.vector.tensor_add(tot[:], prodp[:], rsq[:])
    tot2 = consts.tile([128, 12], f32)
    nc.vector.tensor_add(tot2[:], tot[:], btm_sb)
    outv = consts.tile([128, 3], f32)
    tot_v = bass.AP(
        tensor=tot2[:].tensor,
        offset=tot2[:].offset,
        ap=[tot2[:].ap[0], [1, 3], [3, NB]],
    )
    nc.vector.reduce_sum(outv[:], tot_v, axis=mybir.AxisListType.X)
    nc.sync.dma_start(out=out[0:pr, :], in_=outv[0:pr, :])


_NC_CACHE = {}


def _get_program(ipc=IPC, jt=448):
    key = (ipc, jt)
    if key not in _NC_CACHE:
        _NC_CACHE[key] = build_program(ipc, jt)
    return _NC_CACHE[key]


def _compute_jt(me):
    """Number of leading j columns that cover every unmasked atom,
    rounded up to 16. The standard mask (last 64 padded) gives 448."""
    u = ~np.asarray(me)
    if not u.any():
        return 512
    maxj = int(np.max(np.nonzero(u.any(0))[0])) + 1
    return min(512, -(-maxj // 16) * 16)


def make_in_maps(inputs, jt):
    """Host-side shard + preprocess. Returns per-core input dicts."""
    pf = np.asarray(inputs["pointwise_features"], np.float32)
    rf = np.asarray(inputs["relative_features"], np.float32)
    pb = np.asarray(inputs["pointwise_basis"], np.float32)
    rb = np.asarray(inputs["relative_basis"], np.float32)
    me = np.asarray(inputs["masked_elements"])
    u = (~me).astype(np.float32)  # [B, N]
    A = u.sum(-1).astype(np.float32)  # [B]

    relb2 = np.asarray(inputs["rel_b2"], np.float32)
    assert np.all(relb2 == 0.0), (
        "kernel's flipped layer-2 assumes rel_b2 == 0 (true for this problem)"
    )
    W3 = np.ascontiguousarray(inputs["rel_W3"], np.float32)  # [128, 4]
    b3 = np.asarray(inputs["rel_b3"], np.float32)  # [4]

    cstb = np.ascontiguousarray(inputs["rel_W2"], np.float32).astype(
        ml_dtypes.bfloat16
    )
    w1f = np.ascontiguousarray(
        8.0 * np.asarray(inputs["rel_W1"], np.float32)
    ).astype(ml_dtypes.float8_e4m3fn)

    # shared part of the f32 constant blob
    base = np.zeros((128, 675), np.float32)
    # w3q[32*r + k*3 + c, h] = W3[h, k]: W3 replicated into each of the
    # four 32-partition groups that hold one i-row's [12, h] MT output
    base[:, 0:128] = np.tile(
        np.pad(np.repeat(W3.T, 3, axis=0), ((0, 20), (0, 0))), (4, 1)
    )
    base[:, 128:256] = np.eye(128, dtype=np.float32)
    base[:, 256:384] = np.asarray(inputs["pw_W1"], np.float32)
    base[:, 384:512] = np.asarray(inputs["pw_W2"], np.float32)
    base[:, 512:516] = np.asarray(inputs["pw_W3"], np.float32)
    base[:, 516] = np.asarray(inputs["pw_b1"], np.float32)
    base[:, 517] = np.asarray(inputs["pw_b2"], np.float32)
    base[:, 518:522] = np.asarray(inputs["pw_b3"], np.float32)[None, :]
    base[:, 674] = np.asarray(inputs["rel_b1"], np.float32)

    in_maps = []
    for core in range(NCORES):
        b = core // (NCORES // B)
        i0 = (core % (NCORES // B)) * IPC
        sl = slice(i0, i0 + IPC)
        us = u[b] / (A[b] * A[b])  # [N]
        rbw = rb[b, sl].reshape(IPC, N, 12) * us[None, :, None]
        rbm = (
            rbw.reshape(IPC, NCH, 128, 12)
            .transpose(2, 0, 1, 3)  # [p, IPC, ch, 12] - partition major
            .reshape(128, IPC, NCH * 12)
            .astype(np.float32)
        )
        cstf = base.copy()
        cstf[0:IPC, 522:534] = pb[b, sl].reshape(IPC, 12) / A[b]
        cstf[0:IPC, 534:662] = pf[b, sl]
        # b3 contribution of the j-term, folded on host
        cstf[0:IPC, 662:674] = rbw.sum(1) * np.repeat(b3, 3)[None, :]
        m = {
            "xr": np.ascontiguousarray(
                rf[b, sl, 0:jt, :].transpose(2, 0, 1)
            ).astype(ml_dtypes.float8_e4m3fn),
            "w1f": w1f,
            "rbm": np.ascontiguousarray(rbm).astype(ml_dtypes.bfloat16),
            "cstf": cstf,
            "cstb": cstb,
        }
        in_maps.append(m)
    return in_maps


def kernel(**inputs):
    from concourse.bass_utils import run_bass_kernel_spmd

    jt = _compute_jt(inputs["masked_elements"])
    nc = _get_program(IPC, jt)
    in_maps = make_in_maps(inputs, jt)
    res = run_bass_kernel_spmd(nc, in_maps, core_ids=list(range(NCORES)))
    outs = np.stack([res.results[c]["out"] for c in range(NCORES)])  # [8,128,3]
    return outs.reshape(B, N, 3).astype(np.float32)


# revision 41
# speedup vs baseline: 1.0072x; 1.0072x over previous
"""Trainium2 Bass kernel for DenseEquivariantShiftModule (v2: pipelined pairs).

shift[b,i,c] = ( sum_k pb[b,i,k,c]*ps[b,i,k]
               + (1/A_b) sum_k sum_j u[b,j]*rb[b,i,j,k,c]*rs[b,i,j,k] ) / A_b
where ps = MLP_pw(pointwise_features), rs = MLP_rel(relative_features),
u = ~masked, A_b = sum_j u[b,j].

Sharding: B*N = 1024 "i" rows split across 8 cores (128 rows each, each
core within one batch element).

v2 schedule (per PAIR of i-rows, software-pipelined across stages):
  PE   : L1(p) [2 matmuls, 448 moving cols each, W1 stationary]
         L2(p-1) [8 chunk matmuls, H1 chunks stationary, W2 moving]
         MT(p-2) [8 matmuls, us-weighted rb stationary, H2 moving]
  ACT  : relu1(p) batched over the pair  (PSUM->SBUF, bias b1)
  DVE  : relu2(p-1) batched over the pair; mt*W3 multiply per 4-row group
  GPSIMD: reduce_sum over h per 4-row group -> rall[12, i]
  DMA  : xr trimmed to the 448 unmasked j columns (rb kills masked j).
Final: transpose rall, pointwise MLP, combine, out [128, 3] per core.
"""
import sys

sys.path.insert(0, "/opt/trn_rl_repo")

import ml_dtypes
import numpy as np

import concourse.bass as bass
import concourse.tile as tile
from concourse import masks, mybir

B, N, F, NB = 2, 512, 128, 4
NCORES = 8
IPC = B * N // NCORES  # i-rows per core
NCH = N // 128  # j-chunks per i-row
f32 = mybir.dt.float32
f32r = mybir.dt.float32r
bf16 = mybir.dt.bfloat16
fp8 = mybir.dt.float8e4


def _install_tile_patch():
    """walrus in this container accepts only 1 sem wait per CTRL
    instruction; TileContext's tail drain carries one per touched
    processor. Split them across SP NOPs."""
    import re

    import bass_rust
    from concourse.vector_clock import ScopedClock

    def _patched(self, tick_clock, wait_clock):
        gc = tick_clock.global_clock
        vals = eval(re.match(r"VectorClock\((\[.*\])\)", repr(gc)).group(1))
        for i, v in enumerate(vals):
            if v <= 0:
                continue
            sub = [0] * len(vals)
            sub[i] = v
            nop = self.nc.sync.nop(nofuse=True, hint="drain_wait_split")
            wait_clock.add_sem_waits(
                nop.ins, ScopedClock({None: bass_rust.VectorClock(sub)})
            )
        self.nc.sync.drain()
        self.nc.all_engine_barrier()
        assert self.sems is not None
        popped = self.nc._tile_sem_poison_stack.pop()
        assert popped is self._sem_poison
        self.nc.clear_and_free_semaphores(list(self.sems.allocated().values()))
        self.nc.all_engine_barrier()

    tile.TileContext._drain_and_barrier = _patched


def _split_multi_waits(nc):
    """This walrus build accepts a single sem wait per instruction.
    Move extra waits onto same-engine NOPs inserted just before the
    owning instruction (engine streams execute in block order, so the
    NOP's wait blocks the engine exactly as the fused wait would)."""
    import bass_rust

    n = 0
    for f in nc.m.functions:
        for bb in f.blocks:
            insts = bb.instructions
            i = 0
            while i < len(insts):
                ins = insts[i]
                si = ins.sync_info
                if si is not None and si.on_wait and len(si.on_wait) > 1:
                    waits = list(si.on_wait)
                    updates = list(si.on_update) if si.on_update else []
                    for w in waits[:-1]:
                        nop = mybir.InstNoOp(
                            name=f"I-waitsplit-{n}", ins=[], outs=[]
                        )
                        n += 1
                        nop.engine = ins.engine
                        nop.sync_info = bass_rust.SyncInfo(
                            on_wait=[w], on_update=[]
                        )
                        insts.insert(i, nop)
                        i += 1
                    ins.sync_info = bass_rust.SyncInfo(
                        on_wait=[waits[-1]], on_update=updates
                    )
                i += 1
    return n


def build_program(ipc=IPC, jt=448, split_waits=True):
    _install_tile_patch()
    nc = bass.Bass()
    xr = nc.dram_tensor("xr", [F, ipc, jt], fp8, kind="ExternalInput")
    w1f = nc.dram_tensor("w1f", [F, 128], bf16, kind="ExternalInput")
    rbm = nc.dram_tensor("rbm", [128, ipc, NCH * 12], bf16, kind="ExternalInput")
    # all f32 constants in one blob (single DMA, fat packets):
    # w3q 0:128 | ident 128:256 | pw1 256:384 | pw2 384:512 | pw3 512:516 |
    # pb1 516 | pb2 517 | pb3 518:522 | pbp 522:534 | xp 534:662 |
    # btm 662:674 | b1 674
    cstf = nc.dram_tensor("cstf", [128, 675], f32, kind="ExternalInput")
    cstb = nc.dram_tensor("cstb", [128, 128], bf16, kind="ExternalInput")
    out = nc.dram_tensor("out", [ipc, 3], f32, kind="ExternalOutput")

    from contextlib import ExitStack

    with tile.TileContext(nc) as tc:
        with ExitStack() as ctx:
            _kernel_body(ctx, tc, ipc, jt, xr, w1f, rbm, cstf, cstb, out)
    if split_waits:
        _split_multi_waits(nc)
    return nc


def _kernel_body(ctx, tc, ipc, jt, xr, w1f, rbm, cstf, cstb, out):
    nc = tc.nc
    Relu = mybir.ActivationFunctionType.Relu
    Copy = mybir.ActivationFunctionType.Copy

    consts = ctx.enter_context(tc.tile_pool(name="consts", bufs=1))
    xtpool = ctx.enter_context(tc.tile_pool(name="x", bufs=3))
    h1pool = ctx.enter_context(tc.tile_pool(name="h1", bufs=3))
    h2pool = ctx.enter_context(tc.tile_pool(name="h2", bufs=3))
    smallpool = ctx.enter_context(tc.tile_pool(name="small", bufs=4))
    ps_xt = ctx.enter_context(tc.tile_pool(name="ps_xt", bufs=1, space="PSUM"))
    ps_h1 = ctx.enter_context(tc.tile_pool(name="ps_h1", bufs=2, space="PSUM"))
    ps_h2 = ctx.enter_context(tc.tile_pool(name="ps_h2", bufs=2, space="PSUM"))
    ps_mt = ctx.enter_context(tc.tile_pool(name="ps_mt", bufs=1, space="PSUM"))

    assert ipc % 4 == 0
    NP = ipc // 2  # pairs
    nquads = (ipc + 3) // 4
    pr = min(ipc, 128)

    # xt quads first on the sync queue: the first L1 depends on quad 0, so
    # nothing may queue ahead of it
    xtq = {}

    def issue_quad(q):
        t = xtpool.tile([128, 4, jt], fp8, tag="xts")
        nc.sync.dma_start(out=t[:], in_=xr[:, 4 * q : 4 * q + 4, :])
        xtq[q] = t

    issue_quad(0)
    w1s_t = consts.tile([128, 128], bf16)
    nc.sync.dma_start(out=w1s_t[:], in_=w1f[:])
    cstb_sb = consts.tile([128, 128], bf16)
    nc.sync.dma_start(out=cstb_sb[:], in_=cstb[:])
    if nquads > 1:
        issue_quad(1)
    # rb (host pre-transposed to partition-major, 12KB contiguous runs)
    rb_all = consts.tile([128, ipc, NCH * 12], bf16)
    half = ipc // 2
    nc.gpsimd.dma_start(out=rb_all[:, 0:half, :], in_=rbm[:, 0:half, :])
    nc.gpsimd.dma_start(out=rb_all[:, half:ipc, :], in_=rbm[:, half:ipc, :])
    cstf_sb = consts.tile([128, 675], f32)
    nc.sync.dma_start(out=cstf_sb[:], in_=cstf[:])

    w1s = w1s_t[:]
    w2s = cstb_sb[:, 0:128]
    w3qs = cstf_sb[:, 0:128]
    ident = cstf_sb[:, 128:256]
    pw1s = cstf_sb[:, 256:384]
    pw2s = cstf_sb[:, 384:512]
    pw3s = cstf_sb[:, 512:516]
    pb1s = cstf_sb[:, 516:517]
    pb2s = cstf_sb[:, 517:518]
    pb3s = cstf_sb[:, 518:522]
    pbps = cstf_sb[:, 522:534]
    xp_sb = cstf_sb[:, 534:662]
    btm_sb = cstf_sb[:, 662:674]
    b1s = cstf_sb[:, 674:675]

    # rall2[32*r + kc, g] = j-reduced, W3-contracted result for row 4g+r
    rall2 = consts.tile([128, ipc // 4], f32)

    # zero the unwritten j-tail of the rotating h1 SBUF buffers once so the
    # (finite-garbage) tail columns can never inject NaN downstream
    h1sb_boot = []
    for _ in range(3):
        t = h1pool.tile([128, 2, 512], bf16, tag="h1s")
        if jt < 512:
            nc.vector.memset(t[:, :, jt:512], 0.0)
        h1sb_boot.append(t)

    st_h1sb = {}  # pair -> h1 sbuf tile
    st_h2ps = {}
    st_h2sb = {}
    mtps = {}  # group -> psum tile

    for p in range(NP + 2):
        # ---- stage A: DMA prefetch + L1 + relu1 for pair p ----
        if p < NP:
            q = p // 2
            if p % 2 == 0 and q + 2 < nquads:
                issue_quad(q + 2)
            xt = xtq[q]
            if p < 3:
                h1_sb = h1sb_boot[p]
            else:
                h1_sb = h1pool.tile([128, 2, 512], bf16, tag="h1s")
            for r in range(2):
                a = 2 * (p % 2) + r
                h1_ps = ps_h1.tile([128, 512], f32, tag="h1")
                nc.tensor.matmul(h1_ps[:, 0:jt], w1s, xt[:, a, :])
                # relu1 per-row on ACT; W1 is uploaded as 8*W1 in fp8,
                # the 1/8 rides the activation's free scale
                nc.scalar.activation(
                    h1_sb[:, r, 0:jt], h1_ps[:, 0:jt], Relu, bias=b1s,
                    scale=0.125,
                )
            if p % 2 == 1 and (q - 1) in xtq:
                del xtq[q - 1]
            st_h1sb[p] = h1_sb

        # ---- stage B: L2 + relu2 for pair p-1 ----
        pb_ = p - 1
        if 0 <= pb_ < NP:
            h1_sb = st_h1sb[pb_]
            h2_ps = ps_h2.tile([128, 2, NCH, 128], f32, tag="h2")
            for r in range(2):
                for c in range(NCH):
                    nc.tensor.matmul(
                        h2_ps[:, r, c, :],
                        h1_sb[:, r, c * 128 : (c + 1) * 128],
                        w2s,
                    )
            h2_sb = h2pool.tile([128, 2, NCH, 128], bf16, tag="h2s")
            # relu2 pair-batched on DVE (rel_b2 == 0)
            nc.vector.tensor_scalar(
                h2_sb[:].rearrange("p a c h -> p (a c h)"),
                h2_ps[:].rearrange("p a c h -> p (a c h)"),
                scalar1=0.0,
                scalar2=None,
                op0=mybir.AluOpType.max,
            )
            st_h2ps[pb_] = h2_ps
            st_h2sb[pb_] = h2_sb
            del st_h1sb[pb_]

        # ---- stage C: MT + group close for pair p-2 ----
        # 4 consecutive rows land in one PSUM bank at partition offsets
        # 0/32/64/96 (tile_position column groups); one fused TTR then
        # multiplies by W3 and reduces over h for all 4 rows at once.
        pc_ = p - 2
        if 0 <= pc_ < NP:
            h2_sb = st_h2sb[pc_]
            for r in range(2):
                ii = 2 * pc_ + r
                i4 = ii % 4
                g = ii // 4
                if i4 == 0:
                    mt_ps_new = ps_mt.tile([128, 128], f32, tag="mt")
                    mtps[g] = mt_ps_new
                mt_ps = mtps[g]
                for c in range(NCH):
                    nc.tensor.matmul(
                        mt_ps[32 * i4 : 32 * i4 + 12, :],
                        rb_all[:, ii, c * 12 : (c + 1) * 12],
                        h2_sb[:, r, c, :],
                        start=(c == 0),
                        stop=(c == NCH - 1),
                        tile_position=(0, 32 * i4),
                    )
                if i4 == 3:
                    # fused (mt * w3) + h-sum for 4 packed rows in one op
                    tmp = smallpool.tile([128, 128], f32, tag="tmp")
                    nc.vector.scalar_tensor_tensor(
                        tmp[:],
                        mt_ps[:],
                        1.0,
                        w3qs,
                        op0=mybir.AluOpType.mult,
                        op1=mybir.AluOpType.mult,
                        accum_out=rall2[:, g : g + 1],
                    )
            del st_h2sb[pc_]
            del st_h2ps[pc_]

        # ---- one-shot pointwise MLP, slotted into loop slack right after
        # group 1 closed (its ps_mt buffer is free until group 2 opens) ----
        if p == 5:
            xtp_ps = ps_xt.tile([128, 128], f32, tag="xt")
            nc.tensor.transpose(xtp_ps[:], xp_sb, ident)
            xtp_sb = xtpool.tile([128, 128], f32, tag="xts")
            nc.scalar.activation(xtp_sb[:], xtp_ps[:], Copy)
            h1p_ps = ps_h1.tile([128, 128], f32, tag="h1")
            nc.tensor.matmul(h1p_ps[:], pw1s, xtp_sb[:])
            h1p_sb = h1pool.tile([128, 128], f32, tag="h1s")
            nc.scalar.activation(h1p_sb[:], h1p_ps[:], Relu, bias=pb1s)
            h2p_ps = ps_h2.tile([128, 128], f32, tag="h2")
            nc.tensor.matmul(h2p_ps[:], pw2s, h1p_sb[:])
            h2p_sb = h2pool.tile([128, 128], f32, tag="h2s")
            nc.scalar.activation(h2p_sb[:], h2p_ps[:], Relu, bias=pb2s)
            psc_ps = ps_mt.tile([128, NB], f32, tag="mt")
            nc.tensor.matmul(psc_ps[:], h2p_sb[:], pw3s)
            psc_sb = consts.tile([128, NB], f32)
            nc.vector.tensor_add(psc_sb[:], psc_ps[:], pb3s)

    # unpack rall2 [32r+kc, g] -> rall [kc, 4g+r]
    rall = consts.tile([12, 128], f32)
    for r in range(4):
        dst = bass.AP(
            tensor=rall[:].tensor,
            offset=rall[:].offset + r,
            ap=[rall[:].ap[0], [4, ipc // 4]],
        )
        nc.vector.tensor_copy(dst, rall2[32 * r : 32 * r + 12, :])

    # transpose R [12, i] -> [i, 12]
    rsq_ps = ps_xt.tile([128, 128], f32, tag="xt")
    nc.tensor.transpose(rsq_ps[0:pr, 0:12], rall[0:12, 0:pr],
                        cstf_sb[0:12, 128:140])
    rsq = consts.tile([128, 12], f32)
    if pr < 128:
        nc.vector.memset(rsq[:], 0.0)
    nc.scalar.activation(rsq[0:pr, :], rsq_ps[0:pr, 0:12], Copy)

    # tot[i, kc] = rsq + pbp*ps_bcast + btm ; out = sum_k tot
    prodp = consts.tile([128, 12], f32)
    pb_v = pbps.rearrange("p (k c) -> p k c", k=NB)
    ps_v = bass.AP(
        tensor=psc_sb[:].tensor,
        offset=psc_sb[:].offset,
        ap=[psc_sb[:].ap[0], [1, NB], [0, 3]],
    )
    prodp_v = prodp[:].rearrange("p (k c) -> p k c", k=NB)
    nc.vector.tensor_mul(prodp_v, pb_v, ps_v)
    tot = consts.tile([128, 12], f32)
    nc.vector.tensor_add(tot[:], prodp[:], rsq[:])
    tot2 = consts.tile([128, 12], f32)
    nc.vector.tensor_add(tot2[:], tot[:], btm_sb)
    outv = consts.tile([128, 3], f32)
    tot_v = bass.AP(
        tensor=tot2[:].tensor,
        offset=tot2[:].offset,
        ap=[tot2[:].ap[0], [1, 3], [3, NB]],
    )
    nc.vector.reduce_sum(outv[:], tot_v, axis=mybir.AxisListType.X)
    nc.sync.dma_start(out=out[0:pr, :], in_=outv[0:pr, :])


_NC_CACHE = {}


def _get_program(ipc=IPC, jt=448):
    key = (ipc, jt)
    if key not in _NC_CACHE:
        _NC_CACHE[key] = build_program(ipc, jt)
    return _NC_CACHE[key]


def _compute_jt(me):
    """Number of leading j columns that cover every unmasked atom,
    rounded up to 16. The standard mask (last 64 padded) gives 448."""
    u = ~np.asarray(me)
    if not u.any():
        return 512
    maxj = int(np.max(np.nonzero(u.any(0))[0])) + 1
    return min(512, -(-maxj // 16) * 16)


def make_in_maps(inputs, jt):
    """Host-side shard + preprocess. Returns per-core input dicts."""
    pf = np.asarray(inputs["pointwise_features"], np.float32)
    rf = np.asarray(inputs["relative_features"], np.float32)
    pb = np.asarray(inputs["pointwise_basis"], np.float32)
    rb = np.asarray(inputs["relative_basis"], np.float32)
    me = np.asarray(inputs["masked_elements"])
    u = (~me).astype(np.float32)  # [B, N]
    A = u.sum(-1).astype(np.float32)  # [B]

    relb2 = np.asarray(inputs["rel_b2"], np.float32)
    assert np.all(relb2 == 0.0), (
        "kernel's flipped layer-2 assumes rel_b2 == 0 (true for this problem)"
    )
    W3 = np.ascontiguousarray(inputs["rel_W3"], np.float32)  # [128, 4]
    b3 = np.asarray(inputs["rel_b3"], np.float32)  # [4]

    cstb = np.ascontiguousarray(inputs["rel_W2"], np.float32).astype(
        ml_dtypes.bfloat16
    )
    w1f = np.ascontiguousarray(
        8.0 * np.asarray(inputs["rel_W1"], np.float32)
    ).astype(ml_dtypes.bfloat16)

    # shared part of the f32 constant blob
    base = np.zeros((128, 675), np.float32)
    # w3q[32*r + k*3 + c, h] = W3[h, k]: W3 replicated into each of the
    # four 32-partition groups that hold one i-row's [12, h] MT output
    base[:, 0:128] = np.tile(
        np.pad(np.repeat(W3.T, 3, axis=0), ((0, 20), (0, 0))), (4, 1)
    )
    base[:, 128:256] = np.eye(128, dtype=np.float32)
    base[:, 256:384] = np.asarray(inputs["pw_W1"], np.float32)
    base[:, 384:512] = np.asarray(inputs["pw_W2"], np.float32)
    base[:, 512:516] = np.asarray(inputs["pw_W3"], np.float32)
    base[:, 516] = np.asarray(inputs["pw_b1"], np.float32)
    base[:, 517] = np.asarray(inputs["pw_b2"], np.float32)
    base[:, 518:522] = np.asarray(inputs["pw_b3"], np.float32)[None, :]
    base[:, 674] = np.asarray(inputs["rel_b1"], np.float32)

    in_maps = []
    for core in range(NCORES):
        b = core // (NCORES // B)
        i0 = (core % (NCORES // B)) * IPC
        sl = slice(i0, i0 + IPC)
        us = u[b] / (A[b] * A[b])  # [N]
        rbw = rb[b, sl].reshape(IPC, N, 12) * us[None, :, None]
        rbm = (
            rbw.reshape(IPC, NCH, 128, 12)
            .transpose(2, 0, 1, 3)  # [p, IPC, ch, 12] - partition major
            .reshape(128, IPC, NCH * 12)
            .astype(np.float32)
        )
        cstf = base.copy()
        cstf[0:IPC, 522:534] = pb[b, sl].reshape(IPC, 12) / A[b]
        cstf[0:IPC, 534:662] = pf[b, sl]
        # b3 contribution of the j-term, folded on host
        cstf[0:IPC, 662:674] = rbw.sum(1) * np.repeat(b3, 3)[None, :]
        m = {
            "xr": np.ascontiguousarray(
                rf[b, sl, 0:jt, :].transpose(2, 0, 1)
            ).astype(ml_dtypes.float8_e4m3fn),
            "w1f": w1f,
            "rbm": np.ascontiguousarray(rbm).astype(ml_dtypes.bfloat16),
            "cstf": cstf,
            "cstb": cstb,
        }
        in_maps.append(m)
    return in_maps


def kernel(**inputs):
    from concourse.bass_utils import run_bass_kernel_spmd

    jt = _compute_jt(inputs["masked_elements"])
    nc = _get_program(IPC, jt)
    in_maps = make_in_maps(inputs, jt)
    res = run_bass_kernel_spmd(nc, in_maps, core_ids=list(range(NCORES)))
    outs = np.stack([res.results[c]["out"] for c in range(NCORES)])  # [8,128,3]
    return outs.reshape(B, N, 3).astype(np.float32)


# revision 42
# speedup vs baseline: 1.1739x; 1.1656x over previous
"""Trainium2 Bass kernel for DenseEquivariantShiftModule (v2: pipelined pairs).

shift[b,i,c] = ( sum_k pb[b,i,k,c]*ps[b,i,k]
               + (1/A_b) sum_k sum_j u[b,j]*rb[b,i,j,k,c]*rs[b,i,j,k] ) / A_b
where ps = MLP_pw(pointwise_features), rs = MLP_rel(relative_features),
u = ~masked, A_b = sum_j u[b,j].

Sharding: B*N = 1024 "i" rows split across 8 cores (128 rows each, each
core within one batch element).

v2 schedule (per PAIR of i-rows, software-pipelined across stages):
  PE   : L1(p) [2 matmuls, 448 moving cols each, W1 stationary]
         L2(p-1) [8 chunk matmuls, H1 chunks stationary, W2 moving]
         MT(p-2) [8 matmuls, us-weighted rb stationary, H2 moving]
  ACT  : relu1(p) batched over the pair  (PSUM->SBUF, bias b1)
  DVE  : relu2(p-1) batched over the pair; mt*W3 multiply per 4-row group
  GPSIMD: reduce_sum over h per 4-row group -> rall[12, i]
  DMA  : xr trimmed to the 448 unmasked j columns (rb kills masked j).
Final: transpose rall, pointwise MLP, combine, out [128, 3] per core.
"""
import sys

sys.path.insert(0, "/opt/trn_rl_repo")

import ml_dtypes
import numpy as np

import concourse.bass as bass
import concourse.tile as tile
from concourse import masks, mybir

B, N, F, NB = 2, 512, 128, 4
NCORES = 8
IPC = B * N // NCORES  # i-rows per core
NCH = N // 128  # j-chunks per i-row
f32 = mybir.dt.float32
f32r = mybir.dt.float32r
bf16 = mybir.dt.bfloat16
fp8 = mybir.dt.float8e4


def _install_tile_patch():
    """walrus in this container accepts only 1 sem wait per CTRL
    instruction; TileContext's tail drain carries one per touched
    processor. Split them across SP NOPs."""
    import re

    import bass_rust
    from concourse.vector_clock import ScopedClock

    def _patched(self, tick_clock, wait_clock):
        gc = tick_clock.global_clock
        vals = eval(re.match(r"VectorClock\((\[.*\])\)", repr(gc)).group(1))
        for i, v in enumerate(vals):
            if v <= 0:
                continue
            sub = [0] * len(vals)
            sub[i] = v
            nop = self.nc.sync.nop(nofuse=True, hint="drain_wait_split")
            wait_clock.add_sem_waits(
                nop.ins, ScopedClock({None: bass_rust.VectorClock(sub)})
            )
        self.nc.sync.drain()
        self.nc.all_engine_barrier()
        assert self.sems is not None
        popped = self.nc._tile_sem_poison_stack.pop()
        assert popped is self._sem_poison
        self.nc.clear_and_free_semaphores(list(self.sems.allocated().values()))
        self.nc.all_engine_barrier()

    tile.TileContext._drain_and_barrier = _patched


def _split_multi_waits(nc):
    """This walrus build accepts a single sem wait per instruction.
    Move extra waits onto same-engine NOPs inserted just before the
    owning instruction (engine streams execute in block order, so the
    NOP's wait blocks the engine exactly as the fused wait would)."""
    import bass_rust

    n = 0
    for f in nc.m.functions:
        for bb in f.blocks:
            insts = bb.instructions
            i = 0
            while i < len(insts):
                ins = insts[i]
                si = ins.sync_info
                if si is not None and si.on_wait and len(si.on_wait) > 1:
                    waits = list(si.on_wait)
                    updates = list(si.on_update) if si.on_update else []
                    for w in waits[:-1]:
                        nop = mybir.InstNoOp(
                            name=f"I-waitsplit-{n}", ins=[], outs=[]
                        )
                        n += 1
                        nop.engine = ins.engine
                        nop.sync_info = bass_rust.SyncInfo(
                            on_wait=[w], on_update=[]
                        )
                        insts.insert(i, nop)
                        i += 1
                    ins.sync_info = bass_rust.SyncInfo(
                        on_wait=[waits[-1]], on_update=updates
                    )
                i += 1
    return n


def build_program(ipc=IPC, jt=448, split_waits=True):
    _install_tile_patch()
    nc = bass.Bass()
    xr = nc.dram_tensor("xr", [F, ipc, jt], bf16, kind="ExternalInput")
    w1f = nc.dram_tensor("w1f", [F, 128], bf16, kind="ExternalInput")
    rbm = nc.dram_tensor("rbm", [128, ipc, NCH * 12], bf16, kind="ExternalInput")
    # all f32 constants in one blob (single DMA, fat packets):
    # w3q 0:128 | ident 128:256 | pw1 256:384 | pw2 384:512 | pw3 512:516 |
    # pb1 516 | pb2 517 | pb3 518:522 | pbp 522:534 | xp 534:662 |
    # btm 662:674 | b1 674
    cstf = nc.dram_tensor("cstf", [128, 675], f32, kind="ExternalInput")
    cstb = nc.dram_tensor("cstb", [128, 128], bf16, kind="ExternalInput")
    out = nc.dram_tensor("out", [ipc, 3], f32, kind="ExternalOutput")

    from contextlib import ExitStack

    with tile.TileContext(nc) as tc:
        with ExitStack() as ctx:
            _kernel_body(ctx, tc, ipc, jt, xr, w1f, rbm, cstf, cstb, out)
    if split_waits:
        _split_multi_waits(nc)
    return nc


def _kernel_body(ctx, tc, ipc, jt, xr, w1f, rbm, cstf, cstb, out):
    nc = tc.nc
    Relu = mybir.ActivationFunctionType.Relu
    Copy = mybir.ActivationFunctionType.Copy

    consts = ctx.enter_context(tc.tile_pool(name="consts", bufs=1))
    xtpool = ctx.enter_context(tc.tile_pool(name="x", bufs=3))
    h1pool = ctx.enter_context(tc.tile_pool(name="h1", bufs=3))
    h2pool = ctx.enter_context(tc.tile_pool(name="h2", bufs=3))
    smallpool = ctx.enter_context(tc.tile_pool(name="small", bufs=4))
    ps_xt = ctx.enter_context(tc.tile_pool(name="ps_xt", bufs=1, space="PSUM"))
    ps_h1 = ctx.enter_context(tc.tile_pool(name="ps_h1", bufs=2, space="PSUM"))
    ps_h2 = ctx.enter_context(tc.tile_pool(name="ps_h2", bufs=2, space="PSUM"))
    ps_mt = ctx.enter_context(tc.tile_pool(name="ps_mt", bufs=1, space="PSUM"))

    assert ipc % 4 == 0
    NP = ipc // 2  # pairs
    nquads = (ipc + 3) // 4
    pr = min(ipc, 128)

    # xt quads first on the sync queue: the first L1 depends on quad 0, so
    # nothing may queue ahead of it
    xtq = {}

    def issue_quad(q):
        t = xtpool.tile([128, 4, jt], bf16, tag="xts")
        nc.sync.dma_start(out=t[:], in_=xr[:, 4 * q : 4 * q + 4, :])
        xtq[q] = t

    issue_quad(0)
    w1s_t = consts.tile([128, 128], bf16)
    nc.sync.dma_start(out=w1s_t[:], in_=w1f[:])
    cstb_sb = consts.tile([128, 128], bf16)
    nc.sync.dma_start(out=cstb_sb[:], in_=cstb[:])
    if nquads > 1:
        issue_quad(1)
    # rb (host pre-transposed to partition-major, 12KB contiguous runs)
    rb_all = consts.tile([128, ipc, NCH * 12], bf16)
    half = ipc // 2
    nc.gpsimd.dma_start(out=rb_all[:, 0:half, :], in_=rbm[:, 0:half, :])
    nc.gpsimd.dma_start(out=rb_all[:, half:ipc, :], in_=rbm[:, half:ipc, :])
    cstf_sb = consts.tile([128, 675], f32)
    nc.sync.dma_start(out=cstf_sb[:], in_=cstf[:])

    w1s = w1s_t[:]
    w2s = cstb_sb[:, 0:128]
    w3qs = cstf_sb[:, 0:128]
    ident = cstf_sb[:, 128:256]
    pw1s = cstf_sb[:, 256:384]
    pw2s = cstf_sb[:, 384:512]
    pw3s = cstf_sb[:, 512:516]
    pb1s = cstf_sb[:, 516:517]
    pb2s = cstf_sb[:, 517:518]
    pb3s = cstf_sb[:, 518:522]
    pbps = cstf_sb[:, 522:534]
    xp_sb = cstf_sb[:, 534:662]
    btm_sb = cstf_sb[:, 662:674]
    b1s = cstf_sb[:, 674:675]

    # rall2[32*r + kc, g] = j-reduced, W3-contracted result for row 4g+r
    rall2 = consts.tile([128, ipc // 4], f32)

    # zero the unwritten j-tail of the rotating h1 SBUF buffers once so the
    # (finite-garbage) tail columns can never inject NaN downstream
    h1sb_boot = []
    for _ in range(3):
        t = h1pool.tile([128, 2, 512], bf16, tag="h1s")
        if jt < 512:
            nc.vector.memset(t[:, :, jt:512], 0.0)
        h1sb_boot.append(t)

    st_h1sb = {}  # pair -> h1 sbuf tile
    st_h2ps = {}
    st_h2sb = {}
    mtps = {}  # group -> psum tile

    for p in range(NP + 2):
        # ---- stage A: DMA prefetch + L1 + relu1 for pair p ----
        if p < NP:
            q = p // 2
            if p % 2 == 0 and q + 2 < nquads:
                issue_quad(q + 2)
            xt = xtq[q]
            if p < 3:
                h1_sb = h1sb_boot[p]
            else:
                h1_sb = h1pool.tile([128, 2, 512], bf16, tag="h1s")
            for r in range(2):
                a = 2 * (p % 2) + r
                h1_ps = ps_h1.tile([128, 512], f32, tag="h1")
                nc.tensor.matmul(h1_ps[:, 0:jt], w1s, xt[:, a, :])
                # relu1 per-row on ACT; W1 is uploaded as 8*W1 in fp8,
                # the 1/8 rides the activation's free scale
                nc.scalar.activation(
                    h1_sb[:, r, 0:jt], h1_ps[:, 0:jt], Relu, bias=b1s,
                    scale=0.125,
                )
            if p % 2 == 1 and (q - 1) in xtq:
                del xtq[q - 1]
            st_h1sb[p] = h1_sb

        # ---- stage B: L2 + relu2 for pair p-1 ----
        pb_ = p - 1
        if 0 <= pb_ < NP:
            h1_sb = st_h1sb[pb_]
            h2_ps = ps_h2.tile([128, 2, NCH, 128], f32, tag="h2")
            for r in range(2):
                for c in range(NCH):
                    nc.tensor.matmul(
                        h2_ps[:, r, c, :],
                        h1_sb[:, r, c * 128 : (c + 1) * 128],
                        w2s,
                    )
            h2_sb = h2pool.tile([128, 2, NCH, 128], bf16, tag="h2s")
            # relu2 pair-batched on DVE (rel_b2 == 0)
            nc.vector.tensor_scalar(
                h2_sb[:].rearrange("p a c h -> p (a c h)"),
                h2_ps[:].rearrange("p a c h -> p (a c h)"),
                scalar1=0.0,
                scalar2=None,
                op0=mybir.AluOpType.max,
            )
            st_h2ps[pb_] = h2_ps
            st_h2sb[pb_] = h2_sb
            del st_h1sb[pb_]

        # ---- stage C: MT + group close for pair p-2 ----
        # 4 consecutive rows land in one PSUM bank at partition offsets
        # 0/32/64/96 (tile_position column groups); one fused TTR then
        # multiplies by W3 and reduces over h for all 4 rows at once.
        pc_ = p - 2
        if 0 <= pc_ < NP:
            h2_sb = st_h2sb[pc_]
            for r in range(2):
                ii = 2 * pc_ + r
                i4 = ii % 4
                g = ii // 4
                if i4 == 0:
                    mt_ps_new = ps_mt.tile([128, 128], f32, tag="mt")
                    mtps[g] = mt_ps_new
                mt_ps = mtps[g]
                for c in range(NCH):
                    nc.tensor.matmul(
                        mt_ps[32 * i4 : 32 * i4 + 12, :],
                        rb_all[:, ii, c * 12 : (c + 1) * 12],
                        h2_sb[:, r, c, :],
                        start=(c == 0),
                        stop=(c == NCH - 1),
                        tile_position=(0, 32 * i4),
                    )
                if i4 == 3:
                    # fused (mt * w3) + h-sum for 4 packed rows in one op
                    tmp = smallpool.tile([128, 128], f32, tag="tmp")
                    nc.vector.scalar_tensor_tensor(
                        tmp[:],
                        mt_ps[:],
                        1.0,
                        w3qs,
                        op0=mybir.AluOpType.mult,
                        op1=mybir.AluOpType.mult,
                        accum_out=rall2[:, g : g + 1],
                    )
            del st_h2sb[pc_]
            del st_h2ps[pc_]

        # ---- one-shot pointwise MLP, slotted into loop slack right after
        # group 1 closed (its ps_mt buffer is free until group 2 opens) ----
        if p == 5:
            xtp_ps = ps_xt.tile([128, 128], f32, tag="xt")
            nc.tensor.transpose(xtp_ps[:], xp_sb, ident)
            xtp_sb = xtpool.tile([128, 128], f32, tag="xts")
            nc.scalar.activation(xtp_sb[:], xtp_ps[:], Copy)
            h1p_ps = ps_h1.tile([128, 128], f32, tag="h1")
            nc.tensor.matmul(h1p_ps[:], pw1s, xtp_sb[:])
            h1p_sb = h1pool.tile([128, 128], f32, tag="h1s")
            nc.scalar.activation(h1p_sb[:], h1p_ps[:], Relu, bias=pb1s)
            h2p_ps = ps_h2.tile([128, 128], f32, tag="h2")
            nc.tensor.matmul(h2p_ps[:], pw2s, h1p_sb[:])
            h2p_sb = h2pool.tile([128, 128], f32, tag="h2s")
            nc.scalar.activation(h2p_sb[:], h2p_ps[:], Relu, bias=pb2s)
            psc_ps = ps_mt.tile([128, NB], f32, tag="mt")
            nc.tensor.matmul(psc_ps[:], h2p_sb[:], pw3s)
            psc_sb = consts.tile([128, NB], f32)
            nc.vector.tensor_add(psc_sb[:], psc_ps[:], pb3s)

    # unpack rall2 [32r+kc, g] -> rall [kc, 4g+r]
    rall = consts.tile([12, 128], f32)
    for r in range(4):
        dst = bass.AP(
            tensor=rall[:].tensor,
            offset=rall[:].offset + r,
            ap=[rall[:].ap[0], [4, ipc // 4]],
        )
        nc.vector.tensor_copy(dst, rall2[32 * r : 32 * r + 12, :])

    # transpose R [12, i] -> [i, 12]
    rsq_ps = ps_xt.tile([128, 128], f32, tag="xt")
    nc.tensor.transpose(rsq_ps[0:pr, 0:12], rall[0:12, 0:pr],
                        cstf_sb[0:12, 128:140])
    rsq = consts.tile([128, 12], f32)
    if pr < 128:
        nc.vector.memset(rsq[:], 0.0)
    nc.scalar.activation(rsq[0:pr, :], rsq_ps[0:pr, 0:12], Copy)

    # tot[i, kc] = rsq + pbp*ps_bcast + btm ; out = sum_k tot
    prodp = consts.tile([128, 12], f32)
    pb_v = pbps.rearrange("p (k c) -> p k c", k=NB)
    ps_v = bass.AP(
        tensor=psc_sb[:].tensor,
        offset=psc_sb[:].offset,
        ap=[psc_sb[:].ap[0], [1, NB], [0, 3]],
    )
    prodp_v = prodp[:].rearrange("p (k c) -> p k c", k=NB)
    nc.vector.tensor_mul(prodp_v, pb_v, ps_v)
    tot = consts.tile([128, 12], f32)
    nc.vector.tensor_add(tot[:], prodp[:], rsq[:])
    tot2 = consts.tile([128, 12], f32)
    nc.vector.tensor_add(tot2[:], tot[:], btm_sb)
    outv = consts.tile([128, 3], f32)
    tot_v = bass.AP(
        tensor=tot2[:].tensor,
        offset=tot2[:].offset,
        ap=[tot2[:].ap[0], [1, 3], [3, NB]],
    )
    nc.vector.reduce_sum(outv[:], tot_v, axis=mybir.AxisListType.X)
    nc.sync.dma_start(out=out[0:pr, :], in_=outv[0:pr, :])


_NC_CACHE = {}


def _get_program(ipc=IPC, jt=448):
    key = (ipc, jt)
    if key not in _NC_CACHE:
        _NC_CACHE[key] = build_program(ipc, jt)
    return _NC_CACHE[key]


def _compute_jt(me):
    """Number of leading j columns that cover every unmasked atom,
    rounded up to 16. The standard mask (last 64 padded) gives 448."""
    u = ~np.asarray(me)
    if not u.any():
        return 512
    maxj = int(np.max(np.nonzero(u.any(0))[0])) + 1
    return min(512, -(-maxj // 16) * 16)


def make_in_maps(inputs, jt):
    """Host-side shard + preprocess. Returns per-core input dicts."""
    pf = np.asarray(inputs["pointwise_features"], np.float32)
    rf = np.asarray(inputs["relative_features"], np.float32)
    pb = np.asarray(inputs["pointwise_basis"], np.float32)
    rb = np.asarray(inputs["relative_basis"], np.float32)
    me = np.asarray(inputs["masked_elements"])
    u = (~me).astype(np.float32)  # [B, N]
    A = u.sum(-1).astype(np.float32)  # [B]

    relb2 = np.asarray(inputs["rel_b2"], np.float32)
    assert np.all(relb2 == 0.0), (
        "kernel's flipped layer-2 assumes rel_b2 == 0 (true for this problem)"
    )
    W3 = np.ascontiguousarray(inputs["rel_W3"], np.float32)  # [128, 4]
    b3 = np.asarray(inputs["rel_b3"], np.float32)  # [4]

    cstb = np.ascontiguousarray(inputs["rel_W2"], np.float32).astype(
        ml_dtypes.bfloat16
    )
    w1f = np.ascontiguousarray(
        8.0 * np.asarray(inputs["rel_W1"], np.float32)
    ).astype(ml_dtypes.bfloat16)

    # shared part of the f32 constant blob
    base = np.zeros((128, 675), np.float32)
    # w3q[32*r + k*3 + c, h] = W3[h, k]: W3 replicated into each of the
    # four 32-partition groups that hold one i-row's [12, h] MT output
    base[:, 0:128] = np.tile(
        np.pad(np.repeat(W3.T, 3, axis=0), ((0, 20), (0, 0))), (4, 1)
    )
    base[:, 128:256] = np.eye(128, dtype=np.float32)
    base[:, 256:384] = np.asarray(inputs["pw_W1"], np.float32)
    base[:, 384:512] = np.asarray(inputs["pw_W2"], np.float32)
    base[:, 512:516] = np.asarray(inputs["pw_W3"], np.float32)
    base[:, 516] = np.asarray(inputs["pw_b1"], np.float32)
    base[:, 517] = np.asarray(inputs["pw_b2"], np.float32)
    base[:, 518:522] = np.asarray(inputs["pw_b3"], np.float32)[None, :]
    base[:, 674] = np.asarray(inputs["rel_b1"], np.float32)

    in_maps = []
    for core in range(NCORES):
        b = core // (NCORES // B)
        i0 = (core % (NCORES // B)) * IPC
        sl = slice(i0, i0 + IPC)
        us = u[b] / (A[b] * A[b])  # [N]
        rbw = rb[b, sl].reshape(IPC, N, 12) * us[None, :, None]
        rbm = (
            rbw.reshape(IPC, NCH, 128, 12)
            .transpose(2, 0, 1, 3)  # [p, IPC, ch, 12] - partition major
            .reshape(128, IPC, NCH * 12)
            .astype(np.float32)
        )
        cstf = base.copy()
        cstf[0:IPC, 522:534] = pb[b, sl].reshape(IPC, 12) / A[b]
        cstf[0:IPC, 534:662] = pf[b, sl]
        # b3 contribution of the j-term, folded on host
        cstf[0:IPC, 662:674] = rbw.sum(1) * np.repeat(b3, 3)[None, :]
        m = {
            "xr": np.ascontiguousarray(
                rf[b, sl, 0:jt, :].transpose(2, 0, 1)
            ).astype(ml_dtypes.bfloat16),
            "w1f": w1f,
            "rbm": np.ascontiguousarray(rbm).astype(ml_dtypes.bfloat16),
            "cstf": cstf,
            "cstb": cstb,
        }
        in_maps.append(m)
    return in_maps


def kernel(**inputs):
    from concourse.bass_utils import run_bass_kernel_spmd

    jt = _compute_jt(inputs["masked_elements"])
    nc = _get_program(IPC, jt)
    in_maps = make_in_maps(inputs, jt)
    res = run_bass_kernel_spmd(nc, in_maps, core_ids=list(range(NCORES)))
    outs = np.stack([res.results[c]["out"] for c in range(NCORES)])  # [8,128,3]
    return outs.reshape(B, N, 3).astype(np.float32)
